# revision 23
# baseline (speedup 1.0000x reference)
"""MixQuantActiv on 8 trn2 NeuronCores.

reference semantics (f32, CPU-jax):
    sw = softmax(beta_activ)                      # [3]
    selected = input[:, sel, :, :]                # [32, 8, 56, 56]
    mn, mx = min(selected), max(selected)         # global scalars
    for bit in (2, 4, 8):
        scale_b = (mx - mn) / (2**bit - 1)
        q = clip(round((x - mn) / scale_b), 0, 2**bit - 1)
        activ += (q * scale_b + mn) * sw[i]
    result = input with selected channels replaced by activ
    returns (result, scale_8)

Sharding: batch dim across 8 cores (4 images each). sel / softmax weights are
known at trace time (the program is built per call), so gather/scatter use
static DMA patterns.

Two launches (the global min/max all-reduce is done through the host):
  phase 1 (raw bass, ~10us): each core gathers its selected slice and
     reduces to per-partition (min, max); host max-combines the 8 results.
  phase 2 (Tile): bulk identity copy staged through SBUF on SWDGE (all 16
     SDMA engines), while the quantize pipeline (with every scalar baked as
     an immediate) runs in the copy's shadow; gather/scatter ride the two
     HWDGE rings so they never queue behind the bulk copy.

Numerical contract: every f32 op of the reference is replicated bit-exactly.
round() is the +2^23 RNE trick; division fl(t/s) is reciprocal-multiply plus
a Veltkamp/Markstein correction (validated bit-exact against IEEE division
over 2M+ adversarial samples on hardware), with the residual chain fused
into scalar_tensor_tensor ops computing the negated residual.
"""
import os

import numpy as np

import concourse.bass as bass
import concourse.bacc as bacc
import concourse.mybir as mybir
import concourse.bass_isa as bass_isa
import concourse.tile as tile
from concourse.bass_utils import run_bass_kernel_spmd

SINGLE_KERNEL = os.environ.get("KMODE", "twophase") == "single"

B, C, HW = 32, 256, 56 * 56          # 3136 = 4 * 784
NCORES = 8
BPC = B // NCORES                    # batches per core
BITS = (2, 4, 8)
F = HW // 4                          # 784: free-dim of the packed compute tile
P = 128                              # partitions: 4 quarters * 8 slots * 4 batches
FLAT = BPC * C * HW // P             # 25088: per-partition length of the flat view
CHUNK = 4096                         # staged-copy chunk: [128, 4096] f32 = 2 MB
ABS = 2048                           # flat cols absorbed by phase 1 (1 MB)
DT = mybir.dt.float32
A = mybir.AluOpType
TWO23 = float(2 ** 23)
SPLITC = np.float32(4097.0)          # Veltkamp split constant 2^12 + 1

LAST_RESULT = {}                     # test harness peeks at this


def _build_minmax(sel):
    """Phase 1: gather selected slice -> per-partition (min, max) [128, 2].

    Layout p = 16*slot + 4*batch + quarter: each slot's 3136-element slice
    occupies 16 contiguous partitions, so one DMA per slot (8 total).

    The otherwise-idle SDMA engines also copy the first flat CHUNK of the
    identity copy (staged via SBUF on SWDGE) into `out0`; the host stitches
    it into the final output, so phase 2 skips that chunk.
    """
    nc = bass.Bass()
    x_ext = nc.dram_tensor("x", [BPC, C, HW], DT, kind="ExternalInput")
    mm_ext = nc.dram_tensor("mm", [P, 2], DT, kind="ExternalOutput")
    out0_ext = nc.dram_tensor("out0", [P, ABS], DT, kind="ExternalOutput")

    xq = x_ext[:].rearrange("b c (k f) -> b c k f", k=4)
    x_flat = x_ext[:].rearrange("b c h -> (b c h)").rearrange("(p n) -> p n", p=P)

    with (
        nc.sbuf_tensor([P, F], DT) as X1,
        nc.sbuf_tensor([P, 2], DT) as M2,
        nc.sbuf_tensor([P, ABS], DT) as STG,
        nc.semaphore("dsem") as dsem,
        nc.semaphore("vsem") as vsem,
        nc.semaphore("gsem") as gsem,
        nc.Block() as block,
    ):
        @block.sync
        def _(sync):
            for j, ch in list(enumerate(sel))[:4]:
                sync.dma_start(X1[16 * j:16 * j + 16, :], xq[:, ch]).then_inc(
                    dsem, 16)
            sync.wait_ge(vsem, 1)
            sync.dma_start(mm_ext[:], M2[:]).then_inc(dsem, 16)
            sync.wait_ge(dsem, 9 * 16)

        @block.scalar
        def _(sc):
            for j, ch in list(enumerate(sel))[4:]:
                sc.dma_start(X1[16 * j:16 * j + 16, :], xq[:, ch]).then_inc(
                    dsem, 16)

        @block.vector
        def _(vector):
            vector.wait_ge(dsem, 8 * 16)
            vector.tensor_reduce(M2[:, 0:1], X1[:], axis=mybir.AxisListType.X,
                                 op=A.min)
            vector.tensor_reduce(M2[:, 1:2], X1[:], axis=mybir.AxisListType.X,
                                 op=A.max).then_inc(vsem, 1)

        @block.gpsimd
        def _(gp):
            # 2 MB absorbed copy chunk, pipelined in quarters
            SUB = ABS // 4
            for t in range(4):
                gp.dma_start(STG[:, t * SUB:(t + 1) * SUB],
                             x_flat[:, t * SUB:(t + 1) * SUB]).then_inc(gsem, 16)
            for t in range(4):
                gp.wait_ge(gsem, (t + 1) * 16)
                gp.dma_start(out0_ext[:, t * SUB:(t + 1) * SUB],
                             STG[:, t * SUB:(t + 1) * SUB]).then_inc(gsem, 16)
            gp.wait_ge(gsem, 8 * 16)

    return nc


def _build_main(sel, sw, mn, mx):
    """Phase 2. All scalars are trace-time immediates (f32 python floats)."""
    sel = [int(x) for x in sel]
    uniq = sorted(set(sel))
    slot_of = {}
    for j, ch in enumerate(sel):
        slot_of.setdefault(ch, j)

    # f32 scalar prep, bit-exact with the reference's op order
    mn32, mx32 = np.float32(mn), np.float32(mx)
    rng = np.float32(mx32 - mn32)
    consts = []
    for i, bit in enumerate(BITS):
        qmax = np.float32(2 ** bit - 1)
        s = np.float32(rng / qmax)           # fl(range / qmax)
        r = np.float32(np.float32(1.0) / s)  # fl(1/s)
        c = np.float32(s * SPLITC)
        d = np.float32(c - s)
        s_h = np.float32(c - d)
        s_l = np.float32(s - s_h)
        consts.append(dict(qmax=float(qmax), s=float(s), r=float(r),
                           nr=float(-r), sh=float(s_h), sl=float(s_l),
                           sw=float(np.float32(sw[i]))))
    scale8 = consts[2]["s"]

    nc = bacc.Bacc(None, target_bir_lowering=False, debug=True)
    x_ext = nc.dram_tensor("x", [BPC, C, HW], DT, kind="ExternalInput")
    out_ext = nc.dram_tensor("out", [BPC, C, HW], DT, kind="ExternalOutput")

    xq = x_ext[:].rearrange("b c (k f) -> b c k f", k=4)      # [4,256,4,784]
    oq = out_ext[:].rearrange("b c (k f) -> b c k f", k=4)
    x_flat = x_ext[:].rearrange("b c h -> (b c h)").rearrange("(p n) -> p n", p=P)
    o_flat = out_ext[:].rearrange("b c h -> (b c h)").rearrange("(p n) -> p n", p=P)

    with tile.TileContext(nc) as tc:
        with tc.tile_pool(name="pool", bufs=1) as pool, \
             tc.tile_pool(name="tmp", bufs=1) as tmp, \
             tc.tile_pool(name="stage", bufs=6) as stage:
            # gather selected slice into X [128, 784] on the sync HWDGE ring:
            # partition p = 16*slot + 4*batch + quarter -> each slot is 16
            # contiguous partitions -> one DMA per slot
            X = pool.tile([P, F], DT)
            for j, ch in enumerate(sel):
                nc.sync.dma_start(X[16 * j:16 * j + 16, :], xq[:, ch])

            # bulk identity copy (all channels; scatter overwrites selected
            # ones at the end) staged through SBUF on SWDGE: HBM -> SBUF ->
            # HBM, 2 MB chunks, 100 KB contiguous per partition row.
            # cols [0, ABS) were already copied by phase 1 (host stitches).
            for c0 in range(ABS, FLAT, CHUNK):
                cn = min(CHUNK, FLAT - c0)
                stg = stage.tile([P, CHUNK], DT, name="stg")
                nc.gpsimd.dma_start(stg[:, :cn], x_flat[:, c0:c0 + cn])
                nc.gpsimd.dma_start(o_flat[:, c0:c0 + cn], stg[:, :cn])

            # t = x - mn
            T = pool.tile([P, F], DT)
            nc.vector.tensor_scalar(T[:], X[:], float(mn32), None, A.subtract)

            act = pool.tile([P, F], DT)
            for i, k in enumerate(consts):
                # v = fl(T / s): q0 = T*r; Veltkamp split of q0; negated
                # residual chain; v = q0 + e*r (via e_neg * (-r))
                q0 = tmp.tile([P, F], DT, name="q0")
                nc.vector.tensor_scalar(q0[:], T[:], k["r"], None, A.mult)
                c = tmp.tile([P, F], DT, name="c")
                nc.vector.tensor_scalar(c[:], q0[:], float(SPLITC), None, A.mult)
                d = tmp.tile([P, F], DT, name="d")
                nc.vector.tensor_sub(d[:], c[:], q0[:])
                qh = tmp.tile([P, F], DT, name="qh")
                nc.vector.tensor_sub(qh[:], c[:], d[:])
                ql = tmp.tile([P, F], DT, name="ql")
                nc.vector.tensor_sub(ql[:], q0[:], qh[:])
                e0 = tmp.tile([P, F], DT, name="e0")
                nc.vector.scalar_tensor_tensor(e0[:], qh[:], k["sh"], T[:],
                                               A.mult, A.subtract)  # qh*sh - t
                e1 = tmp.tile([P, F], DT, name="e1")
                nc.vector.scalar_tensor_tensor(e1[:], ql[:], k["sh"], e0[:],
                                               A.mult, A.add)
                e2 = tmp.tile([P, F], DT, name="e2")
                nc.vector.scalar_tensor_tensor(e2[:], qh[:], k["sl"], e1[:],
                                               A.mult, A.add)
                e3 = tmp.tile([P, F], DT, name="e3")
                nc.vector.scalar_tensor_tensor(e3[:], ql[:], k["sl"], e2[:],
                                               A.mult, A.add)       # -e
                v = tmp.tile([P, F], DT, name="v")
                nc.vector.scalar_tensor_tensor(v[:], e3[:], k["nr"], q0[:],
                                               A.mult, A.add)
                # q = min(rne(v), qmax); v >= 0 so the lower clip is a no-op
                w = tmp.tile([P, F], DT, name="w")
                nc.vector.tensor_scalar(w[:], v[:], TWO23, None, A.add)
                q = tmp.tile([P, F], DT, name="q")
                nc.vector.tensor_scalar(q[:], w[:], TWO23, k["qmax"],
                                        A.subtract, A.min)
                # dq = q*s + mn ; act += dq * sw[i]
                dq = tmp.tile([P, F], DT, name="dq")
                nc.vector.tensor_scalar(dq[:], q[:], k["s"], float(mn32),
                                        A.mult, A.add)
                if i == 0:
                    nc.vector.tensor_scalar(act[:], dq[:], k["sw"], None, A.mult)
                else:
                    nc.vector.scalar_tensor_tensor(act[:], dq[:], k["sw"],
                                                   act[:], A.mult, A.add)

            # scatter act back into out on the ACT HWDGE ring (one DMA per
            # unique channel: 16 contiguous partitions -> [4,4,784] dst)
            for ch in uniq:
                p0 = 16 * slot_of[ch]
                nc.scalar.dma_start(oq[:, ch], act[p0:p0 + 16, :])

    nc.compile()
    return nc, scale8


def _build_single(sel, sw):
    """Single-launch variant: min/max via in-kernel AllGather + local max.

    Issue order: gather + reduce + AllGather first (the collective's latency
    hides under the bulk copy), then the staged copy, then the quantize chain
    with per-partition scalar columns, then scatter.
    """
    sel = [int(x) for x in sel]
    uniq = sorted(set(sel))
    slot_of = {}
    for j, ch in enumerate(sel):
        slot_of.setdefault(ch, j)

    nc = bacc.Bacc(None, target_bir_lowering=False, debug=True)
    x_ext = nc.dram_tensor("x", [BPC, C, HW], DT, kind="ExternalInput")
    out_ext = nc.dram_tensor("out", [BPC, C, HW], DT, kind="ExternalOutput")
    scale_ext = nc.dram_tensor("scale", [1, 1], DT, kind="ExternalOutput")

    xq = x_ext[:].rearrange("b c (k f) -> b c k f", k=4)
    oq = out_ext[:].rearrange("b c (k f) -> b c k f", k=4)
    x_flat = x_ext[:].rearrange("b c h -> (b c h)").rearrange("(p n) -> p n", p=P)
    o_flat = out_ext[:].rearrange("b c h -> (b c h)").rearrange("(p n) -> p n", p=P)

    with tile.TileContext(nc) as tc:
        with tc.tile_pool(name="pool", bufs=1) as pool, \
             tc.tile_pool(name="tmp", bufs=2) as tmp, \
             tc.tile_pool(name="stage", bufs=4) as stage, \
             tc.tile_pool(name="dram", bufs=1, space="DRAM") as dram:
            # gather selected slice (one DMA per slot, sync HWDGE ring)
            X = pool.tile([P, F], DT)
            for j, ch in enumerate(sel):
                nc.sync.dma_start(X[16 * j:16 * j + 16, :], xq[:, ch])

            # local (min,max) -> per-partition broadcast -> AllGather -> max
            colmin = pool.tile([P, 1], DT)
            nc.vector.tensor_reduce(colmin[:], X[:], axis=mybir.AxisListType.X,
                                    op=A.min)
            mm2 = pool.tile([P, 2], DT)
            nc.vector.tensor_reduce(mm2[:, 1:2], X[:], axis=mybir.AxisListType.X,
                                    op=A.max)
            nc.vector.tensor_scalar(mm2[:, 0:1], colmin[:], -1.0, None, A.mult)
            pmm = pool.tile([P, 2], DT)
            nc.gpsimd.partition_all_reduce(pmm[:], mm2[:], channels=P,
                                           reduce_op=bass_isa.ReduceOp.max)
            cc_in = dram.tile([P, 2], DT)
            cc_out = dram.tile([NCORES, P, 2], DT, addr_space="Shared")
            nc.sync.dma_start(cc_in[:], pmm[:])
            nc.gpsimd.collective_compute(
                "AllGather", A.bypass, replica_groups=[list(range(NCORES))],
                ins=[cc_in[:]], outs=[cc_out[:]],
            )
            # load as [128, (col, block)] and reduce over blocks
            g = pool.tile([P, 2, NCORES], DT)
            src = cc_out[:].rearrange("b p c -> p c b")
            nc.sync.dma_start(g[:], src)
            bc = pool.tile([P, 2], DT)
            nc.vector.tensor_reduce(bc[:], g[:], axis=mybir.AxisListType.X,
                                    op=A.max)

            # bulk identity copy staged through SBUF on SWDGE
            for c0 in range(0, FLAT, CHUNK):
                cn = min(CHUNK, FLAT - c0)
                stg = stage.tile([P, CHUNK], DT, name="stg")
                nc.gpsimd.dma_start(stg[:, :cn], x_flat[:, c0:c0 + cn])
                nc.gpsimd.dma_start(o_flat[:, c0:c0 + cn], stg[:, :cn])

            mn = pool.tile([P, 1], DT)
            nc.vector.tensor_scalar(mn[:], bc[:, 0:1], -1.0, None, A.mult)
            rng = pool.tile([P, 1], DT)
            nc.vector.tensor_sub(rng[:], bc[:, 1:2], mn[:])

            # t = x - mn
            T = pool.tile([P, F], DT)
            nc.vector.tensor_scalar(T[:], X[:], mn[:, 0:1], None, A.subtract)

            act = pool.tile([P, F], DT)
            s8 = None
            for i, bit in enumerate(BITS):
                qmax = float(2 ** bit - 1)
                r_q = float(np.float32(1.0) / np.float32(qmax))
                nm = f"b{bit}"
                # s = fl(rng/qmax): Markstein; qh*qmax / ql*qmax exact
                q0s = pool.tile([P, 1], DT, name=f"q0s_{nm}")
                nc.vector.tensor_scalar(q0s[:], rng[:], r_q, None, A.mult)
                cs = pool.tile([P, 1], DT, name=f"cs_{nm}")
                nc.vector.tensor_scalar(cs[:], q0s[:], float(SPLITC), None, A.mult)
                dsv = pool.tile([P, 1], DT, name=f"ds_{nm}")
                nc.vector.tensor_sub(dsv[:], cs[:], q0s[:])
                qhs = pool.tile([P, 1], DT, name=f"qhs_{nm}")
                nc.vector.tensor_sub(qhs[:], cs[:], dsv[:])
                qls = pool.tile([P, 1], DT, name=f"qls_{nm}")
                nc.vector.tensor_sub(qls[:], q0s[:], qhs[:])
                esn = pool.tile([P, 1], DT, name=f"esn_{nm}")
                nc.vector.scalar_tensor_tensor(esn[:], qhs[:], qmax, rng[:],
                                               A.mult, A.subtract)
                esn1 = pool.tile([P, 1], DT, name=f"esn1_{nm}")
                nc.vector.scalar_tensor_tensor(esn1[:], qls[:], qmax, esn[:],
                                               A.mult, A.add)
                s_col = pool.tile([P, 1], DT, name=f"s_{nm}")
                nc.vector.scalar_tensor_tensor(s_col[:], esn1[:], -r_q, q0s[:],
                                               A.mult, A.add)
                if bit == 8:
                    s8 = s_col
                r_col = pool.tile([P, 1], DT, name=f"r_{nm}")
                nc.vector.reciprocal(r_col[:], s_col[:])
                nr_col = pool.tile([P, 1], DT, name=f"nr_{nm}")
                nc.vector.tensor_scalar(nr_col[:], r_col[:], -1.0, None, A.mult)
                csp = pool.tile([P, 1], DT, name=f"csp_{nm}")
                nc.vector.tensor_scalar(csp[:], s_col[:], float(SPLITC), None,
                                        A.mult)
                dsp = pool.tile([P, 1], DT, name=f"dsp_{nm}")
                nc.vector.tensor_sub(dsp[:], csp[:], s_col[:])
                sh = pool.tile([P, 1], DT, name=f"sh_{nm}")
                nc.vector.tensor_sub(sh[:], csp[:], dsp[:])
                sl = pool.tile([P, 1], DT, name=f"sl_{nm}")
                nc.vector.tensor_sub(sl[:], s_col[:], sh[:])

                # per-element pipeline with per-partition scalar columns
                q0 = tmp.tile([P, F], DT, name="q0")
                nc.vector.tensor_scalar(q0[:], T[:], r_col[:, 0:1], None, A.mult)
                c = tmp.tile([P, F], DT, name="c")
                nc.vector.tensor_scalar(c[:], q0[:], float(SPLITC), None, A.mult)
                d = tmp.tile([P, F], DT, name="d")
                nc.vector.tensor_sub(d[:], c[:], q0[:])
                qh = tmp.tile([P, F], DT, name="qh")
                nc.vector.tensor_sub(qh[:], c[:], d[:])
                ql = tmp.tile([P, F], DT, name="ql")
                nc.vector.tensor_sub(ql[:], q0[:], qh[:])
                e0 = tmp.tile([P, F], DT, name="e0")
                nc.vector.scalar_tensor_tensor(e0[:], qh[:], sh[:, 0:1], T[:],
                                               A.mult, A.subtract)
                e1 = tmp.tile([P, F], DT, name="e1")
                nc.vector.scalar_tensor_tensor(e1[:], ql[:], sh[:, 0:1], e0[:],
                                               A.mult, A.add)
                e2 = tmp.tile([P, F], DT, name="e2")
                nc.vector.scalar_tensor_tensor(e2[:], qh[:], sl[:, 0:1], e1[:],
                                               A.mult, A.add)
                e3 = tmp.tile([P, F], DT, name="e3")
                nc.vector.scalar_tensor_tensor(e3[:], ql[:], sl[:, 0:1], e2[:],
                                               A.mult, A.add)
                v = tmp.tile([P, F], DT, name="v")
                nc.vector.scalar_tensor_tensor(v[:], e3[:], nr_col[:, 0:1],
                                               q0[:], A.mult, A.add)
                w = tmp.tile([P, F], DT, name="w")
                nc.vector.tensor_scalar(w[:], v[:], TWO23, None, A.add)
                q = tmp.tile([P, F], DT, name="q")
                nc.vector.tensor_scalar(q[:], w[:], TWO23, qmax,
                                        A.subtract, A.min)
                dq = tmp.tile([P, F], DT, name="dq")
                nc.vector.tensor_scalar(dq[:], q[:], s_col[:, 0:1], mn[:, 0:1],
                                        A.mult, A.add)
                if i == 0:
                    nc.vector.tensor_scalar(act[:], dq[:], float(sw[i]), None,
                                            A.mult)
                else:
                    nc.vector.scalar_tensor_tensor(act[:], dq[:], float(sw[i]),
                                                   act[:], A.mult, A.add)

            for ch in uniq:
                p0 = 16 * slot_of[ch]
                nc.scalar.dma_start(oq[:, ch], act[p0:p0 + 16, :])

            nc.sync.dma_start(scale_ext[:], s8[0:1, 0:1])

    nc.compile()
    return nc


def kernel(input, beta_activ, selected_channels):
    input = np.ascontiguousarray(input, dtype=np.float32)
    b = np.asarray(beta_activ, dtype=np.float32)
    # softmax in f32, same op order as jax.nn.softmax on CPU; XLA's f32 exp
    # is near-correctly-rounded, so compute exp in f64 and round
    e = np.exp((b - b.max()).astype(np.float64)).astype(np.float32)
    sw = (e / e.sum(dtype=np.float32)).astype(np.float32)
    sel = np.asarray(selected_channels).astype(np.int64).tolist()

    xr = input.reshape(B, C, HW)
    in_maps = [{"x": np.ascontiguousarray(xr[c * BPC:(c + 1) * BPC])}
               for c in range(NCORES)]
    core_ids = list(range(NCORES))

    if SINGLE_KERNEL:
        nc = _build_single(sel, sw)
        res = run_bass_kernel_spmd(nc, in_maps, core_ids)
        LAST_RESULT["res"] = res
        out = np.concatenate([res.results[c]["out"] for c in core_ids], axis=0)
        return out.reshape(B, C, 56, 56), np.float32(res.results[0]["scale"][0, 0])

    # phase 1: per-core (min, max) of the selected slice; host all-reduce
    nc1 = _build_minmax(sel)
    res1 = run_bass_kernel_spmd(nc1, in_maps, core_ids)
    mms = np.stack([res1.results[c]["mm"] for c in core_ids])   # [8, 128, 2]
    mn = np.float32(mms[:, :, 0].min())
    mx = np.float32(mms[:, :, 1].max())

    # phase 2: copy (chunks 1..) + quantize + scatter
    nc2, scale8 = _build_main(sel, sw, mn, mx)
    res2 = run_bass_kernel_spmd(nc2, in_maps, core_ids)
    LAST_RESULT["res"] = res2

    # stitch: chunk 0 of the flat view comes from phase 1's copy, except the
    # selected-channel slices that phase 2's scatter wrote
    uniq = sorted(set(sel))
    parts = []                       # (partition, col0, col1) inside chunk 0
    for b in range(BPC):
        for ch in uniq:
            s = b * C + ch
            p0, m = s // 8, s % 8
            c0 = m * HW
            if c0 < ABS:
                parts.append((p0, c0, min(c0 + HW, ABS)))
    outs = []
    for c in core_ids:
        f1 = np.array(res2.results[c]["out"]).reshape(P, FLAT)
        f0 = res1.results[c]["out0"]
        saved = [(p0, c0, c1, f1[p0, c0:c1].copy()) for p0, c0, c1 in parts]
        f1[:, :ABS] = f0
        for p0, c0, c1, v in saved:
            f1[p0, c0:c1] = v
        outs.append(f1.reshape(BPC, C, HW))
    out = np.concatenate(outs, axis=0).reshape(B, C, 56, 56)
    return out, np.float32(scale8)


# revision 24
# speedup vs baseline: 1.0680x; 1.0680x over previous
"""MixQuantActiv on 8 trn2 NeuronCores.

reference semantics (f32, CPU-jax):
    sw = softmax(beta_activ)                      # [3]
    selected = input[:, sel, :, :]                # [32, 8, 56, 56]
    mn, mx = min(selected), max(selected)         # global scalars
    for bit in (2, 4, 8):
        scale_b = (mx - mn) / (2**bit - 1)
        q = clip(round((x - mn) / scale_b), 0, 2**bit - 1)
        activ += (q * scale_b + mn) * sw[i]
    result = input with selected channels replaced by activ
    returns (result, scale_8)

Sharding: batch dim across 8 cores (4 images each). sel / softmax weights are
known at trace time (the program is built per call), so gather/scatter use
static DMA patterns.

Two launches (the global min/max all-reduce is done through the host):
  phase 1 (raw bass, ~10us): each core gathers its selected slice and
     reduces to per-partition (min, max); host max-combines the 8 results.
  phase 2 (Tile): bulk identity copy staged through SBUF on SWDGE (all 16
     SDMA engines), while the quantize pipeline (with every scalar baked as
     an immediate) runs in the copy's shadow; gather/scatter ride the two
     HWDGE rings so they never queue behind the bulk copy.

Numerical contract: every f32 op of the reference is replicated bit-exactly.
round() is the +2^23 RNE trick; division fl(t/s) is reciprocal-multiply plus
a Veltkamp/Markstein correction (validated bit-exact against IEEE division
over 2M+ adversarial samples on hardware), with the residual chain fused
into scalar_tensor_tensor ops computing the negated residual.
"""
import os

import numpy as np

import concourse.bass as bass
import concourse.bacc as bacc
import concourse.mybir as mybir
import concourse.bass_isa as bass_isa
import concourse.tile as tile
from concourse.bass_utils import run_bass_kernel_spmd

SINGLE_KERNEL = os.environ.get("KMODE", "twophase") == "single"

B, C, HW = 32, 256, 56 * 56          # 3136 = 4 * 784
NCORES = 8
BPC = B // NCORES                    # batches per core
BITS = (2, 4, 8)
F = HW // 4                          # 784: free-dim of the packed compute tile
P = 128                              # partitions: 4 quarters * 8 slots * 4 batches
FLAT = BPC * C * HW // P             # 25088: per-partition length of the flat view
CHUNK = 4096                         # staged-copy chunk: [128, 4096] f32 = 2 MB
ABS = 3072                           # flat cols absorbed by phase 1 (1.5 MB)
DT = mybir.dt.float32
A = mybir.AluOpType
TWO23 = float(2 ** 23)
SPLITC = np.float32(4097.0)          # Veltkamp split constant 2^12 + 1

LAST_RESULT = {}                     # test harness peeks at this


def _build_minmax(sel):
    """Phase 1: gather selected slice -> per-partition (min, max) [128, 2].

    Layout p = 16*slot + 4*batch + quarter: each slot's 3136-element slice
    occupies 16 contiguous partitions, so one DMA per slot (8 total).

    The otherwise-idle SDMA engines also copy the first flat CHUNK of the
    identity copy (staged via SBUF on SWDGE) into `out0`; the host stitches
    it into the final output, so phase 2 skips that chunk.
    """
    nc = bass.Bass()
    x_ext = nc.dram_tensor("x", [BPC, C, HW], DT, kind="ExternalInput")
    mm_ext = nc.dram_tensor("mm", [P, 2], DT, kind="ExternalOutput")
    out0_ext = nc.dram_tensor("out0", [P, ABS], DT, kind="ExternalOutput")

    xq = x_ext[:].rearrange("b c (k f) -> b c k f", k=4)
    x_flat = x_ext[:].rearrange("b c h -> (b c h)").rearrange("(p n) -> p n", p=P)

    with (
        nc.sbuf_tensor([P, F], DT) as X1,
        nc.sbuf_tensor([P, 2], DT) as M2,
        nc.sbuf_tensor([P, ABS], DT) as STG,
        nc.semaphore("dsem") as dsem,
        nc.semaphore("vsem") as vsem,
        nc.semaphore("gsem") as gsem,
        nc.Block() as block,
    ):
        @block.sync
        def _(sync):
            for j, ch in list(enumerate(sel))[:4]:
                sync.dma_start(X1[16 * j:16 * j + 16, :], xq[:, ch]).then_inc(
                    dsem, 16)
            sync.wait_ge(vsem, 1)
            sync.dma_start(mm_ext[:], M2[:]).then_inc(dsem, 16)
            sync.wait_ge(dsem, 9 * 16)

        @block.scalar
        def _(sc):
            for j, ch in list(enumerate(sel))[4:]:
                sc.dma_start(X1[16 * j:16 * j + 16, :], xq[:, ch]).then_inc(
                    dsem, 16)

        @block.vector
        def _(vector):
            vector.wait_ge(dsem, 8 * 16)
            vector.tensor_reduce(M2[:, 0:1], X1[:], axis=mybir.AxisListType.X,
                                 op=A.min)
            vector.tensor_reduce(M2[:, 1:2], X1[:], axis=mybir.AxisListType.X,
                                 op=A.max).then_inc(vsem, 1)

        @block.gpsimd
        def _(gp):
            # 2 MB absorbed copy chunk, pipelined in quarters
            SUB = ABS // 4
            for t in range(4):
                gp.dma_start(STG[:, t * SUB:(t + 1) * SUB],
                             x_flat[:, t * SUB:(t + 1) * SUB]).then_inc(gsem, 16)
            for t in range(4):
                gp.wait_ge(gsem, (t + 1) * 16)
                gp.dma_start(out0_ext[:, t * SUB:(t + 1) * SUB],
                             STG[:, t * SUB:(t + 1) * SUB]).then_inc(gsem, 16)
            gp.wait_ge(gsem, 8 * 16)

    return nc


def _build_main(sel, sw, mn, mx):
    """Phase 2. All scalars are trace-time immediates (f32 python floats)."""
    sel = [int(x) for x in sel]
    uniq = sorted(set(sel))
    slot_of = {}
    for j, ch in enumerate(sel):
        slot_of.setdefault(ch, j)

    # f32 scalar prep, bit-exact with the reference's op order
    mn32, mx32 = np.float32(mn), np.float32(mx)
    rng = np.float32(mx32 - mn32)
    consts = []
    for i, bit in enumerate(BITS):
        qmax = np.float32(2 ** bit - 1)
        s = np.float32(rng / qmax)           # fl(range / qmax)
        r = np.float32(np.float32(1.0) / s)  # fl(1/s)
        c = np.float32(s * SPLITC)
        d = np.float32(c - s)
        s_h = np.float32(c - d)
        s_l = np.float32(s - s_h)
        consts.append(dict(qmax=float(qmax), s=float(s), r=float(r),
                           nr=float(-r), sh=float(s_h), sl=float(s_l),
                           sw=float(np.float32(sw[i]))))
    scale8 = consts[2]["s"]

    nc = bacc.Bacc(None, target_bir_lowering=False, debug=True)
    x_ext = nc.dram_tensor("x", [BPC, C, HW], DT, kind="ExternalInput")
    out_ext = nc.dram_tensor("out", [BPC, C, HW], DT, kind="ExternalOutput")

    xq = x_ext[:].rearrange("b c (k f) -> b c k f", k=4)      # [4,256,4,784]
    oq = out_ext[:].rearrange("b c (k f) -> b c k f", k=4)
    x_flat = x_ext[:].rearrange("b c h -> (b c h)").rearrange("(p n) -> p n", p=P)
    o_flat = out_ext[:].rearrange("b c h -> (b c h)").rearrange("(p n) -> p n", p=P)

    with tile.TileContext(nc) as tc:
        with tc.tile_pool(name="pool", bufs=1) as pool, \
             tc.tile_pool(name="tmp", bufs=1) as tmp, \
             tc.tile_pool(name="stage", bufs=6) as stage:
            # gather selected slice into X [128, 784] on the sync HWDGE ring:
            # partition p = 16*slot + 4*batch + quarter -> each slot is 16
            # contiguous partitions -> one DMA per slot
            X = pool.tile([P, F], DT)
            for j, ch in enumerate(sel):
                nc.sync.dma_start(X[16 * j:16 * j + 16, :], xq[:, ch])

            # bulk identity copy (all channels; scatter overwrites selected
            # ones at the end) staged through SBUF on SWDGE: HBM -> SBUF ->
            # HBM, 2 MB chunks, 100 KB contiguous per partition row.
            # cols [0, ABS) were already copied by phase 1 (host stitches).
            for c0 in range(ABS, FLAT, CHUNK):
                cn = min(CHUNK, FLAT - c0)
                stg = stage.tile([P, CHUNK], DT, name="stg")
                nc.gpsimd.dma_start(stg[:, :cn], x_flat[:, c0:c0 + cn])
                nc.gpsimd.dma_start(o_flat[:, c0:c0 + cn], stg[:, :cn])

            # t = x - mn
            T = pool.tile([P, F], DT)
            nc.vector.tensor_scalar(T[:], X[:], float(mn32), None, A.subtract)

            act = pool.tile([P, F], DT)
            for i, k in enumerate(consts):
                # v = fl(T / s): q0 = T*r; Veltkamp split of q0; negated
                # residual chain; v = q0 + e*r (via e_neg * (-r))
                q0 = tmp.tile([P, F], DT, name="q0")
                nc.vector.tensor_scalar(q0[:], T[:], k["r"], None, A.mult)
                c = tmp.tile([P, F], DT, name="c")
                nc.vector.tensor_scalar(c[:], q0[:], float(SPLITC), None, A.mult)
                d = tmp.tile([P, F], DT, name="d")
                nc.vector.tensor_sub(d[:], c[:], q0[:])
                qh = tmp.tile([P, F], DT, name="qh")
                nc.vector.tensor_sub(qh[:], c[:], d[:])
                ql = tmp.tile([P, F], DT, name="ql")
                nc.vector.tensor_sub(ql[:], q0[:], qh[:])
                e0 = tmp.tile([P, F], DT, name="e0")
                nc.vector.scalar_tensor_tensor(e0[:], qh[:], k["sh"], T[:],
                                               A.mult, A.subtract)  # qh*sh - t
                e1 = tmp.tile([P, F], DT, name="e1")
                nc.vector.scalar_tensor_tensor(e1[:], ql[:], k["sh"], e0[:],
                                               A.mult, A.add)
                e2 = tmp.tile([P, F], DT, name="e2")
                nc.vector.scalar_tensor_tensor(e2[:], qh[:], k["sl"], e1[:],
                                               A.mult, A.add)
                e3 = tmp.tile([P, F], DT, name="e3")
                nc.vector.scalar_tensor_tensor(e3[:], ql[:], k["sl"], e2[:],
                                               A.mult, A.add)       # -e
                v = tmp.tile([P, F], DT, name="v")
                nc.vector.scalar_tensor_tensor(v[:], e3[:], k["nr"], q0[:],
                                               A.mult, A.add)
                # q = min(rne(v), qmax); v >= 0 so the lower clip is a no-op
                w = tmp.tile([P, F], DT, name="w")
                nc.vector.tensor_scalar(w[:], v[:], TWO23, None, A.add)
                q = tmp.tile([P, F], DT, name="q")
                nc.vector.tensor_scalar(q[:], w[:], TWO23, k["qmax"],
                                        A.subtract, A.min)
                # dq = q*s + mn ; act += dq * sw[i]
                dq = tmp.tile([P, F], DT, name="dq")
                nc.vector.tensor_scalar(dq[:], q[:], k["s"], float(mn32),
                                        A.mult, A.add)
                if i == 0:
                    nc.vector.tensor_scalar(act[:], dq[:], k["sw"], None, A.mult)
                else:
                    nc.vector.scalar_tensor_tensor(act[:], dq[:], k["sw"],
                                                   act[:], A.mult, A.add)

            # scatter act back into out on the ACT HWDGE ring (one DMA per
            # unique channel: 16 contiguous partitions -> [4,4,784] dst)
            for ch in uniq:
                p0 = 16 * slot_of[ch]
                nc.scalar.dma_start(oq[:, ch], act[p0:p0 + 16, :])

    nc.compile()
    return nc, scale8


def _build_single(sel, sw):
    """Single-launch variant: min/max via in-kernel AllGather + local max.

    Issue order: gather + reduce + AllGather first (the collective's latency
    hides under the bulk copy), then the staged copy, then the quantize chain
    with per-partition scalar columns, then scatter.
    """
    sel = [int(x) for x in sel]
    uniq = sorted(set(sel))
    slot_of = {}
    for j, ch in enumerate(sel):
        slot_of.setdefault(ch, j)

    nc = bacc.Bacc(None, target_bir_lowering=False, debug=True)
    x_ext = nc.dram_tensor("x", [BPC, C, HW], DT, kind="ExternalInput")
    out_ext = nc.dram_tensor("out", [BPC, C, HW], DT, kind="ExternalOutput")
    scale_ext = nc.dram_tensor("scale", [1, 1], DT, kind="ExternalOutput")

    xq = x_ext[:].rearrange("b c (k f) -> b c k f", k=4)
    oq = out_ext[:].rearrange("b c (k f) -> b c k f", k=4)
    x_flat = x_ext[:].rearrange("b c h -> (b c h)").rearrange("(p n) -> p n", p=P)
    o_flat = out_ext[:].rearrange("b c h -> (b c h)").rearrange("(p n) -> p n", p=P)

    with tile.TileContext(nc) as tc:
        with tc.tile_pool(name="pool", bufs=1) as pool, \
             tc.tile_pool(name="tmp", bufs=2) as tmp, \
             tc.tile_pool(name="stage", bufs=4) as stage, \
             tc.tile_pool(name="dram", bufs=1, space="DRAM") as dram:
            # gather selected slice (one DMA per slot, sync HWDGE ring)
            X = pool.tile([P, F], DT)
            for j, ch in enumerate(sel):
                nc.sync.dma_start(X[16 * j:16 * j + 16, :], xq[:, ch])

            # local (min,max) -> per-partition broadcast -> AllGather -> max
            colmin = pool.tile([P, 1], DT)
            nc.vector.tensor_reduce(colmin[:], X[:], axis=mybir.AxisListType.X,
                                    op=A.min)
            mm2 = pool.tile([P, 2], DT)
            nc.vector.tensor_reduce(mm2[:, 1:2], X[:], axis=mybir.AxisListType.X,
                                    op=A.max)
            nc.vector.tensor_scalar(mm2[:, 0:1], colmin[:], -1.0, None, A.mult)
            pmm = pool.tile([P, 2], DT)
            nc.gpsimd.partition_all_reduce(pmm[:], mm2[:], channels=P,
                                           reduce_op=bass_isa.ReduceOp.max)
            cc_in = dram.tile([P, 2], DT)
            cc_out = dram.tile([NCORES, P, 2], DT, addr_space="Shared")
            nc.sync.dma_start(cc_in[:], pmm[:])
            nc.gpsimd.collective_compute(
                "AllGather", A.bypass, replica_groups=[list(range(NCORES))],
                ins=[cc_in[:]], outs=[cc_out[:]],
            )
            # load as [128, (col, block)] and reduce over blocks
            g = pool.tile([P, 2, NCORES], DT)
            src = cc_out[:].rearrange("b p c -> p c b")
            nc.sync.dma_start(g[:], src)
            bc = pool.tile([P, 2], DT)
            nc.vector.tensor_reduce(bc[:], g[:], axis=mybir.AxisListType.X,
                                    op=A.max)

            # bulk identity copy staged through SBUF on SWDGE
            for c0 in range(0, FLAT, CHUNK):
                cn = min(CHUNK, FLAT - c0)
                stg = stage.tile([P, CHUNK], DT, name="stg")
                nc.gpsimd.dma_start(stg[:, :cn], x_flat[:, c0:c0 + cn])
                nc.tensor.dma_start(o_flat[:, c0:c0 + cn], stg[:, :cn])

            mn = pool.tile([P, 1], DT)
            nc.vector.tensor_scalar(mn[:], bc[:, 0:1], -1.0, None, A.mult)
            rng = pool.tile([P, 1], DT)
            nc.vector.tensor_sub(rng[:], bc[:, 1:2], mn[:])

            # t = x - mn
            T = pool.tile([P, F], DT)
            nc.vector.tensor_scalar(T[:], X[:], mn[:, 0:1], None, A.subtract)

            act = pool.tile([P, F], DT)
            s8 = None
            for i, bit in enumerate(BITS):
                qmax = float(2 ** bit - 1)
                r_q = float(np.float32(1.0) / np.float32(qmax))
                nm = f"b{bit}"
                # s = fl(rng/qmax): Markstein; qh*qmax / ql*qmax exact
                q0s = pool.tile([P, 1], DT, name=f"q0s_{nm}")
                nc.vector.tensor_scalar(q0s[:], rng[:], r_q, None, A.mult)
                cs = pool.tile([P, 1], DT, name=f"cs_{nm}")
                nc.vector.tensor_scalar(cs[:], q0s[:], float(SPLITC), None, A.mult)
                dsv = pool.tile([P, 1], DT, name=f"ds_{nm}")
                nc.vector.tensor_sub(dsv[:], cs[:], q0s[:])
                qhs = pool.tile([P, 1], DT, name=f"qhs_{nm}")
                nc.vector.tensor_sub(qhs[:], cs[:], dsv[:])
                qls = pool.tile([P, 1], DT, name=f"qls_{nm}")
                nc.vector.tensor_sub(qls[:], q0s[:], qhs[:])
                esn = pool.tile([P, 1], DT, name=f"esn_{nm}")
                nc.vector.scalar_tensor_tensor(esn[:], qhs[:], qmax, rng[:],
                                               A.mult, A.subtract)
                esn1 = pool.tile([P, 1], DT, name=f"esn1_{nm}")
                nc.vector.scalar_tensor_tensor(esn1[:], qls[:], qmax, esn[:],
                                               A.mult, A.add)
                s_col = pool.tile([P, 1], DT, name=f"s_{nm}")
                nc.vector.scalar_tensor_tensor(s_col[:], esn1[:], -r_q, q0s[:],
                                               A.mult, A.add)
                if bit == 8:
                    s8 = s_col
                r_col = pool.tile([P, 1], DT, name=f"r_{nm}")
                nc.vector.reciprocal(r_col[:], s_col[:])
                nr_col = pool.tile([P, 1], DT, name=f"nr_{nm}")
                nc.vector.tensor_scalar(nr_col[:], r_col[:], -1.0, None, A.mult)
                csp = pool.tile([P, 1], DT, name=f"csp_{nm}")
                nc.vector.tensor_scalar(csp[:], s_col[:], float(SPLITC), None,
                                        A.mult)
                dsp = pool.tile([P, 1], DT, name=f"dsp_{nm}")
                nc.vector.tensor_sub(dsp[:], csp[:], s_col[:])
                sh = pool.tile([P, 1], DT, name=f"sh_{nm}")
                nc.vector.tensor_sub(sh[:], csp[:], dsp[:])
                sl = pool.tile([P, 1], DT, name=f"sl_{nm}")
                nc.vector.tensor_sub(sl[:], s_col[:], sh[:])

                # per-element pipeline with per-partition scalar columns
                q0 = tmp.tile([P, F], DT, name="q0")
                nc.vector.tensor_scalar(q0[:], T[:], r_col[:, 0:1], None, A.mult)
                c = tmp.tile([P, F], DT, name="c")
                nc.vector.tensor_scalar(c[:], q0[:], float(SPLITC), None, A.mult)
                d = tmp.tile([P, F], DT, name="d")
                nc.vector.tensor_sub(d[:], c[:], q0[:])
                qh = tmp.tile([P, F], DT, name="qh")
                nc.vector.tensor_sub(qh[:], c[:], d[:])
                ql = tmp.tile([P, F], DT, name="ql")
                nc.vector.tensor_sub(ql[:], q0[:], qh[:])
                e0 = tmp.tile([P, F], DT, name="e0")
                nc.vector.scalar_tensor_tensor(e0[:], qh[:], sh[:, 0:1], T[:],
                                               A.mult, A.subtract)
                e1 = tmp.tile([P, F], DT, name="e1")
                nc.vector.scalar_tensor_tensor(e1[:], ql[:], sh[:, 0:1], e0[:],
                                               A.mult, A.add)
                e2 = tmp.tile([P, F], DT, name="e2")
                nc.vector.scalar_tensor_tensor(e2[:], qh[:], sl[:, 0:1], e1[:],
                                               A.mult, A.add)
                e3 = tmp.tile([P, F], DT, name="e3")
                nc.vector.scalar_tensor_tensor(e3[:], ql[:], sl[:, 0:1], e2[:],
                                               A.mult, A.add)
                v = tmp.tile([P, F], DT, name="v")
                nc.vector.scalar_tensor_tensor(v[:], e3[:], nr_col[:, 0:1],
                                               q0[:], A.mult, A.add)
                w = tmp.tile([P, F], DT, name="w")
                nc.vector.tensor_scalar(w[:], v[:], TWO23, None, A.add)
                q = tmp.tile([P, F], DT, name="q")
                nc.vector.tensor_scalar(q[:], w[:], TWO23, qmax,
                                        A.subtract, A.min)
                dq = tmp.tile([P, F], DT, name="dq")
                nc.vector.tensor_scalar(dq[:], q[:], s_col[:, 0:1], mn[:, 0:1],
                                        A.mult, A.add)
                if i == 0:
                    nc.vector.tensor_scalar(act[:], dq[:], float(sw[i]), None,
                                            A.mult)
                else:
                    nc.vector.scalar_tensor_tensor(act[:], dq[:], float(sw[i]),
                                                   act[:], A.mult, A.add)

            for ch in uniq:
                p0 = 16 * slot_of[ch]
                nc.scalar.dma_start(oq[:, ch], act[p0:p0 + 16, :])

            nc.sync.dma_start(scale_ext[:], s8[0:1, 0:1])

    nc.compile()
    return nc


def kernel(input, beta_activ, selected_channels):
    input = np.ascontiguousarray(input, dtype=np.float32)
    b = np.asarray(beta_activ, dtype=np.float32)
    # softmax in f32, same op order as jax.nn.softmax on CPU; XLA's f32 exp
    # is near-correctly-rounded, so compute exp in f64 and round
    e = np.exp((b - b.max()).astype(np.float64)).astype(np.float32)
    sw = (e / e.sum(dtype=np.float32)).astype(np.float32)
    sel = np.asarray(selected_channels).astype(np.int64).tolist()

    xr = input.reshape(B, C, HW)
    in_maps = [{"x": np.ascontiguousarray(xr[c * BPC:(c + 1) * BPC])}
               for c in range(NCORES)]
    core_ids = list(range(NCORES))

    if SINGLE_KERNEL:
        nc = _build_single(sel, sw)
        res = run_bass_kernel_spmd(nc, in_maps, core_ids)
        LAST_RESULT["res"] = res
        out = np.concatenate([res.results[c]["out"] for c in core_ids], axis=0)
        return out.reshape(B, C, 56, 56), np.float32(res.results[0]["scale"][0, 0])

    # phase 1: per-core (min, max) of the selected slice; host all-reduce
    nc1 = _build_minmax(sel)
    res1 = run_bass_kernel_spmd(nc1, in_maps, core_ids)
    mms = np.stack([res1.results[c]["mm"] for c in core_ids])   # [8, 128, 2]
    mn = np.float32(mms[:, :, 0].min())
    mx = np.float32(mms[:, :, 1].max())

    # phase 2: copy (chunks 1..) + quantize + scatter
    nc2, scale8 = _build_main(sel, sw, mn, mx)
    res2 = run_bass_kernel_spmd(nc2, in_maps, core_ids)
    LAST_RESULT["res"] = res2

    # stitch: chunk 0 of the flat view comes from phase 1's copy, except the
    # selected-channel slices that phase 2's scatter wrote
    uniq = sorted(set(sel))
    parts = []                       # (partition, col0, col1) inside chunk 0
    for b in range(BPC):
        for ch in uniq:
            s = b * C + ch
            p0, m = s // 8, s % 8
            c0 = m * HW
            if c0 < ABS:
                parts.append((p0, c0, min(c0 + HW, ABS)))
    outs = []
    for c in core_ids:
        f1 = np.array(res2.results[c]["out"]).reshape(P, FLAT)
        f0 = res1.results[c]["out0"]
        saved = [(p0, c0, c1, f1[p0, c0:c1].copy()) for p0, c0, c1 in parts]
        f1[:, :ABS] = f0
        for p0, c0, c1, v in saved:
            f1[p0, c0:c1] = v
        outs.append(f1.reshape(BPC, C, HW))
    out = np.concatenate(outs, axis=0).reshape(B, C, 56, 56)
    return out, np.float32(scale8)
